# revision 30
# baseline (speedup 1.0000x reference)
"""Trainium2 Bass kernel for nn_MultiHeadAttention_36051955483000.

Full-shape contract: kernel(**inputs) takes the complete fp32 tensors
(q,k,v: [4,2048,1024]; Wq/Wk/Wv/Wo: [1024,1024]; biases [1024]) and
returns the full [4,2048,1024] fp32 output.

Sharding (8 NeuronCores): core = 2*b + g for batch b in 0..3 and
head-group g in {0,1}. Each core computes 8 of the 16 heads for one
batch, AllGathers the attention output with its pair core, and runs
the output projection for its 512 output features.

Kernel structure (v2 — chunk-outer pipeline):
- The query-chunk loop (8 chunks of 256 queries) is OUTER; all 4 head
  pairs process a chunk before moving on. Q/K/V projections are
  demand-driven: chunk c+1's projection matmuls are interleaved into
  chunk c's attention stream, so there is no serial projection phase.
- Scores are computed transposed (S^T, keys on partitions), head pairs
  ride disjoint PE row groups (contraction 64), exp(0.125 x) runs on
  the scalar engine, masking via tri-multiply on vector.
- PV matmuls use M=128: columns 0:64 of the stationary operand are the
  head's V, columns 64:128 are ones — output rows 64:128 come out as
  the softmax denominator broadcast across 64 partitions for free
  (replaces the old [1,64,512] PE broadcast matmuls).
- Per chunk, once all 4 head pairs finish, the chunk's x^T columns are
  AllGathered pairwise, and the OUT-PROJECTION FOR THAT CHUNK's tokens
  runs one chunk later, fully overlapped. Only the last chunk's AG +
  out-proj are tail.
- Collectives and their dependent cc_out->SBUF loads live alone on the
  gpsimd queue so their waits never block compute queues.
"""

import numpy as np
import ml_dtypes

B, N, D, H = 4, 2048, 1024, 16
DH = D // H            # 64
HG = H // 2            # 8 heads per core
FG = D // 2            # 512 features per head-group
N_CORES = 8
QC = 256               # query-chunk width
NCH = N // QC          # 8 chunks
NKB = N // 128         # 16 key blocks

BF16 = ml_dtypes.bfloat16

_PROG = None
DEBUG_DUMP = False


def _build_program():
    from concourse import bacc, tile, mybir

    f32 = mybir.dt.float32
    bf16 = mybir.dt.bfloat16

    nc = bacc.Bacc("TRN2", target_bir_lowering=False, debug=False,
                   num_devices=N_CORES)

    xqT = nc.dram_tensor("xqT", [D, N], bf16, kind="ExternalInput").ap()
    xkT = nc.dram_tensor("xkT", [D, N], bf16, kind="ExternalInput").ap()
    xvT = nc.dram_tensor("xvT", [D, N], bf16, kind="ExternalInput").ap()
    wqT = nc.dram_tensor("wqT", [D, FG], bf16, kind="ExternalInput").ap()
    wkT = nc.dram_tensor("wkT", [D, FG], bf16, kind="ExternalInput").ap()
    wvT = nc.dram_tensor("wvT", [D, FG], bf16, kind="ExternalInput").ap()
    woT = nc.dram_tensor("woT", [D, FG], bf16, kind="ExternalInput").ap()
    bq2 = nc.dram_tensor("bq2", [128, 4], f32, kind="ExternalInput").ap()
    bk2 = nc.dram_tensor("bk2", [128, 4], f32, kind="ExternalInput").ap()
    tri01 = nc.dram_tensor("tri01", [128, 128], bf16, kind="ExternalInput").ap()
    y = nc.dram_tensor("y", [N, FG], f32, kind="ExternalOutput").ap()
    if DEBUG_DUMP:
        dkt = nc.dram_tensor("dkt", [128, 4 * N], bf16,
                             kind="ExternalOutput").ap()
        dva = nc.dram_tensor("dva", [128, NKB * HG * 128], bf16,
                             kind="ExternalOutput").ap()

    add = mybir.AluOpType.add
    mult = mybir.AluOpType.mult
    Exp = mybir.ActivationFunctionType.Exp

    with tile.TileContext(nc) as tc:
        with (
            tc.tile_pool(name="consts", bufs=1) as consts,
            tc.tile_pool(name="dram", bufs=1, space="DRAM") as dram,
            tc.tile_pool(name="xin", bufs=48) as xin,
        ):
            wq_sb = consts.tile([128, 8 * FG], bf16, tag="wq")
            wk_sb = consts.tile([128, 8 * FG], bf16, tag="wk")
            wv_sb = consts.tile([128, 8 * FG], bf16, tag="wv")
            wo_sb = consts.tile([128, 8 * FG], bf16, tag="wo")
            kt_sb = consts.tile([128, 4 * N], bf16, tag="kt")
            # per (kblock t, head h): 64 V columns then 64 ones columns;
            # PV with this 128-wide stationary operand emits the softmax
            # denominator broadcast over partitions 64:128 for free.
            vaug_sb = consts.tile([128, NKB * HG * 128], bf16, tag="vaug")
            bq_sb = consts.tile([128, 4], f32, tag="bq")
            bk_sb = consts.tile([128, 4], f32, tag="bk")
            tri_sb = consts.tile([128, 128], bf16, tag="tri")

            cc_in = [dram.tile([128, 4 * QC], bf16, name=f"cc_in{c}",
                               tag=f"cci{c}") for c in range(NCH)]
            cc_out = [dram.tile([256, 4 * QC], bf16, name=f"cc_out{c}",
                                tag=f"cco{c}") for c in range(NCH)]

            # small consts on sync; weights on the scalar queue
            nc.sync.dma_start(bq_sb[:], bq2[:])
            nc.sync.dma_start(bk_sb[:], bk2[:])
            nc.sync.dma_start(tri_sb[:], tri01[:])
            for W_sb, WT in ((wq_sb, wqT), (wk_sb, wkT), (wv_sb, wvT),
                             (wo_sb, woT)):
                for db in range(8):
                    nc.scalar.dma_start(W_sb[:, 512 * db:512 * db + 512],
                                        WT[128 * db:128 * db + 128, :])

            # inputs at half-tensor (1024-token) granularity: dma_start
            # issue costs ~0.6-1.8us of sequencer time EACH, so few large
            # loads beat many small ones. Half 0 of Q/K on sync and V on
            # the (idle at start) vector queue, so chunk 0 starts early;
            # half-1 issues are deferred into chunks 0-2 so the per-chunk
            # AllGather input copies are not stuck behind them in FIFO.
            xts = {}
            deferred_loads = {0: [], 1: [], 2: []}
            for half in range(2):
                for nm, XT in (("q", xqT), ("k", xkT), ("v", xvT)):
                    for db in range(8):
                        t = xin.tile([128, N // 2], bf16, tag="xin",
                                     name=f"x{nm}{db}_{half}")
                        src = XT[128 * db:128 * db + 128,
                                 1024 * half:1024 * half + 1024]
                        if half == 0:
                            eng = nc.gpsimd if nm == "v" else nc.sync
                            eng.dma_start(t[:], src)
                        else:
                            when = {"q": 0, "k": 1, "v": 2}[nm]
                            deferred_loads[when].append((t, src))
                        xts[(nm, db, half)] = t

            vaug_v = vaug_sb[:, :].rearrange("p (t h c) -> p t h c",
                                             t=NKB, h=HG, c=128)
            nc.vector.memset(vaug_v[:, :, :, 64:128], 1.0)

            with (
                tc.tile_pool(name="mm", bufs=2, space="PSUM") as mmp,
                tc.tile_pool(name="sg", bufs=2, space="PSUM") as sgp,
                tc.tile_pool(name="otp", bufs=2, space="PSUM") as otp,
                tc.tile_pool(name="pt", bufs=3) as ptp,
                tc.tile_pool(name="ep", bufs=2) as ep,
                tc.tile_pool(name="xo", bufs=2) as xop,
                tc.tile_pool(name="yp", bufs=2) as yp,
                tc.tile_pool(name="qtp", bufs=2) as qtp,
                tc.tile_pool(name="xtp", bufs=2) as xtp,
            ):
                qtiles = {}      # chunk -> [128, 4*QC] Q^T tile
                xtiles = {}      # chunk -> [128, 4*QC] attention-out tile
                def x_slice(nm, db, lo, width):
                    # columns [lo, lo+width) of the full token axis
                    half = lo // 1024
                    assert (lo + width - 1) // 1024 == half
                    return xts[(nm, db, half)][:, lo - 1024 * half:
                                               lo - 1024 * half + width]

                def proj_sets_for_chunk(c):
                    """Emit-ready list of closures; each emits one PSUM
                    accumulation set (8 matmuls + eviction) for chunk c."""
                    sets = []

                    def qk_set(nm, W_sb, bias, e, c=c):
                        def emit():
                            if nm == "q" and c not in qtiles:
                                qtiles[c] = qtp.tile([128, 4 * QC], bf16,
                                                     tag="qt",
                                                     name=f"qt{c}")
                            ps = mmp.tile([128, 512], f32, tag="mm",
                                          name=f"{nm}{e}_{c}")
                            for db in range(8):
                                nc.tensor.matmul(
                                    ps[:, 0:QC],
                                    lhsT=W_sb[:, 512 * db + 128 * e:
                                              512 * db + 128 * e + 128],
                                    rhs=x_slice(nm, db, QC * c, QC),
                                    start=(db == 0), stop=(db == 7))
                            if nm == "q":
                                out_ap = qtiles[c][:, QC * e:QC * e + QC]
                            else:
                                out_ap = kt_sb[:, 2048 * e + QC * c:
                                               2048 * e + QC * c + QC]
                            nc.vector.tensor_scalar(
                                out_ap, ps[:, 0:QC], bias[:, e:e + 1],
                                None, add)
                        return emit

                    def v_set(tb):
                        def emit():
                            ps = mmp.tile([128, 512], f32, tag="mm",
                                          name=f"v{tb}")
                            for db in range(8):
                                nc.tensor.matmul(
                                    ps[:],
                                    lhsT=x_slice("v", db, 128 * tb, 128),
                                    rhs=wv_sb[:, 512 * db:512 * db + 512],
                                    start=(db == 0), stop=(db == 7))
                            nc.vector.tensor_copy(
                                vaug_v[:, tb, :, 0:64],
                                ps[:, :].rearrange("p (h c) -> p h c",
                                                   h=HG, c=64))
                        return emit

                    for e in range(4):
                        sets.append(qk_set("q", wq_sb, bq_sb, e))
                        sets.append(qk_set("k", wk_sb, bk_sb, e))
                    sets.append(v_set(2 * c))
                    sets.append(v_set(2 * c + 1))
                    return sets

                # prologue: chunk 0's projections
                for s in proj_sets_for_chunk(0):
                    s()

                def emit_epilogue(e, c, OT2):
                    # OT2 [128, 2*QC]: rows 0:64 = O^T (head 2e cols 0:QC,
                    # head 2e+1 cols QC:2QC); rows 64:128 = denominators
                    # broadcast. Normalize and write x^T into xtown.
                    # reciprocal_approx_fast is a custom DVE op; feed it a
                    # partition-0-based tile (copy handles the offset read).
                    dsb = ep.tile([64, 2 * QC], f32, tag="dsb",
                                  name=f"dsb{e}_{c}")
                    nc.vector.tensor_copy(dsb[:, :], OT2[64:128, :])
                    bcs = ep.tile([64, 2 * QC], f32, tag="bcs",
                                  name=f"bcs{e}_{c}")
                    nc.vector.reciprocal_approx_fast(bcs[:, :], dsb[:, :])
                    for half in (0, 1):
                        nc.vector.tensor_tensor(
                            xtiles[c][64 * half:64 * half + 64,
                                      QC * e:QC * e + QC],
                            OT2[0:64, QC * half:QC * half + QC],
                            bcs[:, QC * half:QC * half + QC], mult)

                def emit_outproj(c):
                    # chunk c tokens: load gathered x blocks, contract all
                    # 1024 features into the core's 512 output columns.
                    xta = xop.tile([128, 4 * QC], bf16, tag="xo",
                                   name=f"xta{c}")
                    xtb = xop.tile([128, 4 * QC], bf16, tag="xo",
                                   name=f"xtb{c}")
                    # gpsimd queue: sits right after this chunk's AG wait,
                    # so these never block compute queues.
                    nc.gpsimd.dma_start(xta[:], cc_out[c][0:128, :])
                    nc.gpsimd.dma_start(xtb[:], cc_out[c][128:256, :])
                    for tb in range(2):
                        ps = mmp.tile([128, 512], f32, tag="mm",
                                      name=f"op{c}_{tb}")
                        i = 0
                        for xt in (xta, xtb):
                            for e in range(4):
                                nc.tensor.matmul(
                                    ps[:],
                                    lhsT=xt[:, QC * e + 128 * tb:
                                            QC * e + 128 * tb + 128],
                                    rhs=wo_sb[:, 512 * i:512 * i + 512],
                                    start=(i == 0), stop=(i == 7))
                                i += 1
                        ysb = yp.tile([128, 512], f32, tag="ysb",
                                      name=f"y{c}_{tb}")
                        nc.vector.tensor_copy(ysb[:], ps[:])
                        nc.sync.dma_start(
                            y[QC * c + 128 * tb:QC * c + 128 * tb + 128, :],
                            ysb[:])

                # ---- main chunk-outer attention stream ----
                prev = None          # (c, e, js, PT) lagged one group for PV
                ots = {}             # (c, e) -> OT2 psum tile

                def flush_prev(nxt):
                    nonlocal prev
                    if prev is None:
                        prev = nxt
                        return
                    pc, pe, pjs, pPT = prev
                    pOT2 = ots[(pc, pe)]
                    for m, j in enumerate(pjs):
                        for half in (0, 1):
                            nc.tensor.matmul(
                                pOT2[:, QC * half:QC * half + QC],
                                lhsT=vaug_v[:, j, 2 * pe + half, :],
                                rhs=pPT[:, 512 * half + QC * m:
                                        512 * half + QC * m + QC],
                                # one start per PSUM bank: start clears
                                # has_written bank-wide
                                start=(j == 0 and half == 0),
                                stop=(j == 2 * pc + 1),
                                skip_group_check=True)
                    if pjs[-1] == 2 * pc + 1:     # (pc, pe) complete
                        emit_epilogue(pe, pc, pOT2)
                        del ots[(pc, pe)]
                    prev = nxt

                for c in range(NCH):
                    xtiles[c] = xtp.tile([128, 4 * QC], bf16, tag="xt",
                                         name=f"xt{c}")
                    # deferred half-1 input loads (sync queue, after this
                    # chunk's position so AG copies aren't stuck behind)
                    for t, src in deferred_loads.pop(c, ()):
                        nc.sync.dma_start(t[:], src)
                    proj_next = proj_sets_for_chunk(c + 1) if c < NCH - 1 \
                        else []
                    # group stream: two waves of head pairs to bound live
                    # OT2 banks at 2
                    groups = []
                    for wave in ((0, 1), (2, 3)):
                        for gi in range(c + 1):
                            for e in wave:
                                groups.append((e, gi))
                    # spread next chunk's projection sets over this chunk
                    pts = {}
                    if proj_next:
                        step = max(1, len(groups) // len(proj_next))
                        for i in range(len(proj_next)):
                            pts.setdefault(min((i + 1) * step,
                                               len(groups) - 1), []).append(
                                proj_next[i])

                    for idx, (e, gi) in enumerate(groups):
                        js = [2 * gi, 2 * gi + 1]
                        if gi == 0 and (c, e) not in ots:
                            ots[(c, e)] = otp.tile(
                                [128, 2 * QC], f32, tag="OT2",
                                name=f"OT2_{c}_{e}")
                        hb = 2048 * e
                        SG = sgp.tile([128, 4 * QC], f32, tag="SG",
                                      name=f"SG{c}_{e}_{js[0]}")
                        for m, j in enumerate(js):
                            for half in (0, 1):
                                po = 64 * half
                                off = 512 * half + QC * m
                                kt_j = kt_sb[po:po + 64,
                                             hb + 128 * j:hb + 128 * j + 128]
                                if j <= 2 * c:
                                    nc.tensor.matmul(
                                        SG[:, off:off + QC], lhsT=kt_j,
                                        rhs=qtiles[c][po:po + 64,
                                                      QC * e:QC * e + QC],
                                        start=True, stop=True,
                                        skip_group_check=True)
                                else:   # j == 2c+1: front half is dead
                                    nc.tensor.matmul(
                                        SG[:, off + 128:off + QC],
                                        lhsT=kt_j,
                                        rhs=qtiles[c][po:po + 64,
                                                      QC * e + 128:
                                                      QC * e + QC],
                                        start=True, stop=True,
                                        skip_group_check=True)
                        PT = ptp.tile([128, 4 * QC], bf16, tag="PT",
                                      name=f"PT{c}_{e}_{js[0]}")
                        nc.scalar.activation(PT[:, :], SG[:, :], Exp,
                                             scale=0.125)
                        if js[-1] == 2 * c + 1:   # band group: mask on PT
                            for half in (0, 1):
                                off = 512 * half
                                nc.vector.tensor_tensor(
                                    PT[:, off:off + 128],
                                    PT[:, off:off + 128], tri_sb[:], mult)
                                nc.vector.memset(
                                    PT[:, off + QC:off + QC + 128], 0.0)
                                nc.vector.tensor_tensor(
                                    PT[:, off + QC + 128:off + 2 * QC],
                                    PT[:, off + QC + 128:off + 2 * QC],
                                    tri_sb[:], mult)
                        flush_prev((c, e, js, PT))
                        for s in pts.get(idx, ()):
                            s()

                    # drain the lagged PV for this chunk's last group
                    flush_prev(None)
                    prev = None

                    # chunk complete on all 4 head pairs -> pairwise AG
                    nc.sync.dma_start(cc_in[c][:], xtiles[c][:])
                    nc.gpsimd.collective_compute(
                        "AllGather",
                        mybir.AluOpType.bypass,
                        replica_groups=[[0, 1], [2, 3], [4, 5], [6, 7]],
                        ins=[cc_in[c].opt()],
                        outs=[cc_out[c].opt()],
                    )
                    # out-projection lagged two chunks behind its AG so the
                    # PE queue never waits on collective latency mid-stream
                    if c >= 2:
                        emit_outproj(c - 2)
                emit_outproj(NCH - 2)
                emit_outproj(NCH - 1)
                if DEBUG_DUMP:
                    nc.sync.dma_start(dkt[:], kt_sb[:])
                    nc.sync.dma_start(dva[:], vaug_sb[:])

    nc.compile()
    return nc


def _program():
    global _PROG
    if _PROG is None:
        _PROG = _build_program()
    return _PROG


def _host_inputs(q, k, v, Wq, bq, Wk, bk, Wv, bv, Wo):
    qb = np.asarray(q, np.float32).astype(BF16)
    kb = np.asarray(k, np.float32).astype(BF16)
    vb = np.asarray(v, np.float32).astype(BF16)
    xqT = [np.ascontiguousarray(qb[b].T) for b in range(B)]
    xkT = [np.ascontiguousarray(kb[b].T) for b in range(B)]
    xvT = [np.ascontiguousarray(vb[b].T) for b in range(B)]

    def wslice(W, g):
        return np.ascontiguousarray(
            np.asarray(W, np.float32)[FG * g:FG * (g + 1), :].T).astype(BF16)

    wqg = [wslice(Wq, g) for g in range(2)]
    wkg = [wslice(Wk, g) for g in range(2)]
    wvg = [wslice(Wv, g) for g in range(2)]

    def woslice(g):
        # AllGather output rows are rank-ordered (rank 0 = head-group 0
        # for both cores of a pair), so woT rows stay in natural order.
        return np.ascontiguousarray(
            np.asarray(Wo, np.float32)[FG * g:FG * (g + 1), :].T).astype(BF16)

    wog = [woslice(g) for g in range(2)]

    def bslice(bvec, g):
        return np.ascontiguousarray(
            np.asarray(bvec, np.float32)[FG * g:FG * (g + 1)]
            .reshape(4, 128).T)

    bqg = [bslice(bq, g) for g in range(2)]
    bkg = [bslice(bk, g) for g in range(2)]

    kk, qq = np.meshgrid(np.arange(128), np.arange(128), indexing="ij")
    tri = np.where(kk <= qq, 1.0, 0.0).astype(BF16)

    in_maps = []
    for core in range(N_CORES):
        b, g = core // 2, core % 2
        in_maps.append({
            "xqT": xqT[b], "xkT": xkT[b], "xvT": xvT[b],
            "wqT": wqg[g], "wkT": wkg[g], "wvT": wvg[g], "woT": wog[g],
            "bq2": bqg[g], "bk2": bkg[g], "tri01": tri,
        })
    return in_maps


def run_sharded(in_maps, trace=False, trace_kwargs=None):
    from concourse.bass_utils import run_bass_kernel_spmd
    nc = _program()
    return run_bass_kernel_spmd(nc, in_maps, core_ids=list(range(N_CORES)),
                                trace=trace, trace_kwargs=trace_kwargs or {})


def kernel(q, k, v, Wq, bq, Wk, bk, Wv, bv, Wo):
    in_maps = _host_inputs(q, k, v, Wq, bq, Wk, bk, Wv, bv, Wo)
    res = run_sharded(in_maps)
    out = np.empty((B, N, D), np.float32)
    for b in range(B):
        out[b, :, 0:FG] = res.results[2 * b]["y"]
        out[b, :, FG:D] = res.results[2 * b + 1]["y"]
    return out


# revision 36
# speedup vs baseline: 1.1813x; 1.1813x over previous
"""Trainium2 Bass kernel for nn_MultiHeadAttention_36051955483000.

Full-shape contract: kernel(**inputs) takes the complete fp32 tensors
(q,k,v: [4,2048,1024]; Wq/Wk/Wv/Wo: [1024,1024]; biases [1024]) and
returns the full [4,2048,1024] fp32 output.

Sharding (8 NeuronCores): core = 2*b + g for batch b in 0..3 and
head-group g in {0,1}. Each core computes 8 of the 16 heads for one
batch, AllGathers the attention output with its pair core, and runs
the output projection for its 512 output features.

Kernel structure (v4 — chunk-outer pipeline):
- The query-chunk loop (8 chunks of 256 queries) is OUTER; all 4 head
  pairs process a chunk before moving on. Q/K/V projections are
  demand-driven, emitted per chunk-PAIR at N=512 and interleaved into
  the previous chunks' attention stream: no serial projection phase.
- Scores are computed transposed (S^T, keys on partitions), head pairs
  ride disjoint PE row groups (contraction 64), exp(0.125 x) runs on
  the scalar engine, causal masking via tri-multiply on vector.
- PV matmuls use M=128: columns 0:64 of the stationary operand are the
  head's V, columns 64:128 are ones — output rows 64:128 come out as
  the softmax denominator broadcast across 64 partitions for free.
  The band block's dead front queries are skipped, not memset.
- Per chunk, once all 4 head pairs finish, the chunk's x^T is
  AllGathered pairwise; that chunk's out-projection runs two chunks
  later, fully overlapped. Only the last chunk's AG + out-proj trail.
- Queue discipline: collectives + their dependent cc_out loads live on
  the gpsimd queue (their waits block nothing else); DMA issues are
  few and large (dma_start costs ~0.7us of sequencer time each), with
  half-1 input loads deferred into chunks 0-2 so per-chunk AG copies
  are not stuck behind them in sync-queue FIFO order.
"""

import numpy as np
import ml_dtypes

B, N, D, H = 4, 2048, 1024, 16
DH = D // H            # 64
HG = H // 2            # 8 heads per core
FG = D // 2            # 512 features per head-group
N_CORES = 8
QC = 256               # query-chunk width
NCH = N // QC          # 8 chunks
NKB = N // 128         # 16 key blocks

BF16 = ml_dtypes.bfloat16

_PROG = None
DEBUG_DUMP = False


def _build_program():
    from concourse import bacc, tile, mybir

    f32 = mybir.dt.float32
    bf16 = mybir.dt.bfloat16

    nc = bacc.Bacc("TRN2", target_bir_lowering=False, debug=False,
                   num_devices=N_CORES)

    xqT = nc.dram_tensor("xqT", [D, N], bf16, kind="ExternalInput").ap()
    xkT = nc.dram_tensor("xkT", [D, N], bf16, kind="ExternalInput").ap()
    xvT = nc.dram_tensor("xvT", [D, N], bf16, kind="ExternalInput").ap()
    wqT = nc.dram_tensor("wqT", [D, FG], bf16, kind="ExternalInput").ap()
    wkT = nc.dram_tensor("wkT", [D, FG], bf16, kind="ExternalInput").ap()
    wvT = nc.dram_tensor("wvT", [D, FG], bf16, kind="ExternalInput").ap()
    woT = nc.dram_tensor("woT", [D, FG], bf16, kind="ExternalInput").ap()
    bq2 = nc.dram_tensor("bq2", [128, 4], f32, kind="ExternalInput").ap()
    bk2 = nc.dram_tensor("bk2", [128, 4], f32, kind="ExternalInput").ap()
    tri01 = nc.dram_tensor("tri01", [128, 128], bf16, kind="ExternalInput").ap()
    y = nc.dram_tensor("y", [N, FG], bf16, kind="ExternalOutput").ap()
    if DEBUG_DUMP:
        dkt = nc.dram_tensor("dkt", [128, 4 * N], bf16,
                             kind="ExternalOutput").ap()
        dva = nc.dram_tensor("dva", [128, NKB * HG * 128], bf16,
                             kind="ExternalOutput").ap()

    add = mybir.AluOpType.add
    mult = mybir.AluOpType.mult
    Exp = mybir.ActivationFunctionType.Exp

    with tile.TileContext(nc) as tc:
        with (
            tc.tile_pool(name="consts", bufs=1) as consts,
            tc.tile_pool(name="dram", bufs=1, space="DRAM") as dram,
            tc.tile_pool(name="xin", bufs=48) as xin,
        ):
            wq_sb = consts.tile([128, 8 * FG], bf16, tag="wq")
            wk_sb = consts.tile([128, 8 * FG], bf16, tag="wk")
            wv_sb = consts.tile([128, 8 * FG], bf16, tag="wv")
            wo_sb = consts.tile([128, 8 * FG], bf16, tag="wo")
            kt_sb = consts.tile([128, 4 * N], bf16, tag="kt")
            # per (kblock t, head h): 64 V columns then 64 ones columns;
            # PV with this 128-wide stationary operand emits the softmax
            # denominator broadcast over partitions 64:128 for free.
            vaug_sb = consts.tile([128, NKB * HG * 128], bf16, tag="vaug")
            bq_sb = consts.tile([128, 4], f32, tag="bq")
            bk_sb = consts.tile([128, 4], f32, tag="bk")
            tri_sb = consts.tile([128, 128], bf16, tag="tri")

            cc_in = [dram.tile([128, 4 * QC], bf16, name=f"cc_in{c}",
                               tag=f"cci{c}") for c in range(NCH)]
            cc_out = [dram.tile([256, 4 * QC], bf16, name=f"cc_out{c}",
                                tag=f"cco{c}") for c in range(NCH)]

            # small consts on sync; weights on the scalar queue
            nc.sync.dma_start(bq_sb[:], bq2[:])
            nc.sync.dma_start(bk_sb[:], bk2[:])
            nc.sync.dma_start(tri_sb[:], tri01[:])
            for W_sb, WT in ((wq_sb, wqT), (wk_sb, wkT), (wv_sb, wvT),
                             (wo_sb, woT)):
                for db in range(8):
                    nc.scalar.dma_start(W_sb[:, 512 * db:512 * db + 512],
                                        WT[128 * db:128 * db + 128, :])

            # inputs at half-tensor (1024-token) granularity; half 0 of
            # Q/K on sync and V on the (idle at start) gpsimd queue so
            # chunk 0 starts early; half-1 issues deferred into chunks
            # 0-2 so AG copies aren't stuck behind them in FIFO.
            xts = {}
            deferred_loads = {0: [], 1: [], 2: []}
            for half in range(2):
                for nm, XT in (("q", xqT), ("k", xkT), ("v", xvT)):
                    for db in range(8):
                        t = xin.tile([128, N // 2], bf16, tag="xin",
                                     name=f"x{nm}{db}_{half}")
                        src = XT[128 * db:128 * db + 128,
                                 1024 * half:1024 * half + 1024]
                        if half == 0:
                            eng = nc.gpsimd if nm == "v" else nc.sync
                            eng.dma_start(t[:], src)
                        else:
                            when = {"q": 0, "k": 1, "v": 2}[nm]
                            deferred_loads[when].append((t, src))
                        xts[(nm, db, half)] = t

            vaug_v = vaug_sb[:, :].rearrange("p (t h c) -> p t h c",
                                             t=NKB, h=HG, c=128)
            nc.vector.memset(vaug_v[:, :, :, 64:128], 1.0)

            with (
                tc.tile_pool(name="mm", bufs=2, space="PSUM") as mmp,
                tc.tile_pool(name="sg", bufs=2, space="PSUM") as sgp,
                tc.tile_pool(name="otp", bufs=2, space="PSUM") as otp,
                tc.tile_pool(name="pt", bufs=2) as ptp,
                tc.tile_pool(name="ep", bufs=2) as ep,
                tc.tile_pool(name="xo", bufs=2) as xop,
                tc.tile_pool(name="yp", bufs=2) as yp,
                tc.tile_pool(name="qtp", bufs=2) as qtp,
                tc.tile_pool(name="xtp", bufs=2) as xtp,
            ):
                qtiles = {}      # chunk-pair -> [128, 4*512] Q^T tile
                xtiles = {}      # chunk -> [128, 4*QC] attention-out tile

                def x_slice(nm, db, lo, width):
                    # columns [lo, lo+width) of the full token axis
                    half = lo // 1024
                    assert (lo + width - 1) // 1024 == half
                    return xts[(nm, db, half)][:, lo - 1024 * half:
                                               lo - 1024 * half + width]

                def qk_set(nm, W_sb, bias, e, p):
                    # Q/K projection for head-pair e, chunk pair p
                    # (tokens [512p, 512p+512)), N=512
                    def emit():
                        if nm == "q" and p not in qtiles:
                            qtiles[p] = qtp.tile([128, 4 * 512], bf16,
                                                 tag="qt", name=f"qt{p}")
                        ps = mmp.tile([128, 512], f32, tag="mm",
                                      name=f"{nm}{e}_{p}")
                        for db in range(8):
                            nc.tensor.matmul(
                                ps[:],
                                lhsT=W_sb[:, 512 * db + 128 * e:
                                          512 * db + 128 * e + 128],
                                rhs=x_slice(nm, db, 512 * p, 512),
                                start=(db == 0), stop=(db == 7))
                        if nm == "q":
                            out_ap = qtiles[p][:, 512 * e:512 * e + 512]
                        else:
                            out_ap = kt_sb[:, 2048 * e + 512 * p:
                                           2048 * e + 512 * p + 512]
                        nc.vector.tensor_scalar(
                            out_ap, ps[:], bias[:, e:e + 1], None, add)
                    return emit

                def v_set(tb):
                    # V projection for key block tb (128 tokens), N=512
                    def emit():
                        ps = mmp.tile([128, 512], f32, tag="mm",
                                      name=f"v{tb}")
                        for db in range(8):
                            nc.tensor.matmul(
                                ps[:],
                                lhsT=x_slice("v", db, 128 * tb, 128),
                                rhs=wv_sb[:, 512 * db:512 * db + 512],
                                start=(db == 0), stop=(db == 7))
                        nc.vector.tensor_copy(
                            vaug_v[:, tb, :, 0:64],
                            ps[:, :].rearrange("p (h c) -> p h c",
                                               h=HG, c=64))
                    return emit

                def proj_sets_for_pair(p):
                    sets = []
                    for e in range(4):
                        sets.append(qk_set("q", wq_sb, bq_sb, e, p))
                        sets.append(qk_set("k", wk_sb, bk_sb, e, p))
                    for tb in range(4 * p, 4 * p + 4):
                        sets.append(v_set(tb))
                    return sets

                # prologue: just enough of pair 0 for wave A of chunk 0;
                # the rest is interleaved into chunk 0's stream.
                pair0 = proj_sets_for_pair(0)
                # order: q(e0) k(e0) q(e1) k(e1) v(kb0) v(kb1)
                for s in (pair0[0], pair0[1], pair0[2], pair0[3],
                          pair0[8], pair0[9]):
                    s()
                pair0_rest = [pair0[4], pair0[5], pair0[6], pair0[7],
                              pair0[10], pair0[11]]

                def emit_epilogue(e, c, OT2):
                    # OT2 [128, 2*QC]: rows 0:64 = O^T (head 2e cols 0:QC,
                    # head 2e+1 cols QC:2QC); rows 64:128 = denominators
                    # broadcast. Normalize and write x^T.
                    # reciprocal_approx_fast is a custom DVE op; feed it a
                    # partition-0-based tile (copy handles the offset).
                    dsb = ep.tile([64, 2 * QC], f32, tag="dsb",
                                  name=f"dsb{e}_{c}")
                    nc.vector.tensor_copy(dsb[:, :], OT2[64:128, :])
                    bcs = ep.tile([64, 2 * QC], f32, tag="bcs",
                                  name=f"bcs{e}_{c}")
                    nc.vector.reciprocal_approx_fast(bcs[:, :], dsb[:, :])
                    for half in (0, 1):
                        nc.vector.tensor_tensor(
                            xtiles[c][64 * half:64 * half + 64,
                                      QC * e:QC * e + QC],
                            OT2[0:64, QC * half:QC * half + QC],
                            bcs[:, QC * half:QC * half + QC], mult)

                def emit_outproj(c):
                    # chunk c tokens: load gathered x blocks (gpsimd queue,
                    # right after that AG's wait -> never blocks compute),
                    # contract all 1024 features into 512 output columns.
                    xta = xop.tile([128, 4 * QC], bf16, tag="xo",
                                   name=f"xta{c}")
                    xtb = xop.tile([128, 4 * QC], bf16, tag="xo",
                                   name=f"xtb{c}")
                    nc.gpsimd.dma_start(xta[:], cc_out[c][0:128, :])
                    nc.gpsimd.dma_start(xtb[:], cc_out[c][128:256, :])
                    for tb in range(2):
                        ps = mmp.tile([128, 512], f32, tag="mm",
                                      name=f"op{c}_{tb}")
                        i = 0
                        for xt in (xta, xtb):
                            for e in range(4):
                                nc.tensor.matmul(
                                    ps[:],
                                    lhsT=xt[:, QC * e + 128 * tb:
                                            QC * e + 128 * tb + 128],
                                    rhs=wo_sb[:, 512 * i:512 * i + 512],
                                    start=(i == 0), stop=(i == 7))
                                i += 1
                        ysb = yp.tile([128, 512], bf16, tag="ysb",
                                      name=f"y{c}_{tb}")
                        nc.vector.tensor_copy(ysb[:], ps[:])
                        nc.sync.dma_start(
                            y[QC * c + 128 * tb:QC * c + 128 * tb + 128, :],
                            ysb[:])

                # ---- main chunk-outer attention stream ----
                prev = None          # (c, e, js, PT) lagged one group
                ots = {}             # (c, e) -> OT2 psum tile

                def flush_prev(nxt):
                    nonlocal prev
                    if prev is None:
                        prev = nxt
                        return
                    pc, pe, pjs, pPT = prev
                    pOT2 = ots[(pc, pe)]
                    for m, j in enumerate(pjs):
                        for half in (0, 1):
                            if j == 2 * pc + 1:
                                # band block: front 128 queries are dead
                                # (never written) — stream only the back
                                rhs = pPT[:, 512 * half + QC * m + 128:
                                          512 * half + QC * m + QC]
                                out = pOT2[:, QC * half + 128:
                                           QC * half + QC]
                            else:
                                rhs = pPT[:, 512 * half + QC * m:
                                          512 * half + QC * m + QC]
                                out = pOT2[:, QC * half:QC * half + QC]
                            nc.tensor.matmul(
                                out, lhsT=vaug_v[:, j, 2 * pe + half, :],
                                rhs=rhs,
                                # one start per PSUM bank: start clears
                                # has_written bank-wide
                                start=(j == 0 and half == 0),
                                stop=(j == 2 * pc + 1),
                                skip_group_check=True)
                    if pjs[-1] == 2 * pc + 1:     # (pc, pe) complete
                        emit_epilogue(pe, pc, pOT2)
                        del ots[(pc, pe)]
                    prev = nxt

                for c in range(NCH):
                    xtiles[c] = xtp.tile([128, 4 * QC], bf16, tag="xt",
                                         name=f"xt{c}")
                    # deferred half-1 input loads (sync queue)
                    for t, src in deferred_loads.pop(c, ()):
                        nc.sync.dma_start(t[:], src)
                    # projection sets to interleave into this chunk: pair
                    # p = chunks (2p, 2p+1), emitted over chunks 2p-2/2p-1
                    pn = []
                    if c == 0:
                        pn = proj_sets_for_pair(1)[0:6]
                    elif c <= 5:
                        p = c // 2 + 1
                        pn = (proj_sets_for_pair(p)[0:6] if c % 2 == 0
                              else proj_sets_for_pair(p)[6:12])
                    groups = []
                    for wave in ((0, 1), (2, 3)):
                        for gi in range(c + 1):
                            for e in wave:
                                groups.append((e, gi))
                    pts = {}
                    if c == 0:
                        # deadline-aware: group (e,0) consumes q/k(e) —
                        # those sets must precede it in PE queue order
                        pts = {0: pair0_rest[0:2],    # q/k(e2) before grp 2
                               1: pair0_rest[2:4],    # q/k(e3) before grp 3
                               3: pair0_rest[4:6] + pn}
                    elif pn:
                        step = max(1, len(groups) // len(pn))
                        for i in range(len(pn)):
                            pts.setdefault(min((i + 1) * step,
                                               len(groups) - 1), []).append(
                                pn[i])

                    for idx, (e, gi) in enumerate(groups):
                        js = [2 * gi, 2 * gi + 1]
                        if gi == 0 and (c, e) not in ots:
                            ots[(c, e)] = otp.tile(
                                [128, 2 * QC], f32, tag="OT2",
                                name=f"OT2_{c}_{e}")
                        SG = sgp.tile([128, 4 * QC], f32, tag="SG",
                                      name=f"SG{c}_{e}_{js[0]}")
                        qt_p = qtiles[c // 2]
                        qoff = 512 * e + QC * (c % 2)
                        for m, j in enumerate(js):
                            for half in (0, 1):
                                po = 64 * half
                                off = 512 * half + QC * m
                                kt_j = kt_sb[po:po + 64,
                                             2048 * e + 128 * j:
                                             2048 * e + 128 * j + 128]
                                if j <= 2 * c:
                                    nc.tensor.matmul(
                                        SG[:, off:off + QC], lhsT=kt_j,
                                        rhs=qt_p[po:po + 64,
                                                 qoff:qoff + QC],
                                        start=True, stop=True,
                                        skip_group_check=True)
                                else:   # j == 2c+1: front half is dead
                                    nc.tensor.matmul(
                                        SG[:, off + 128:off + QC],
                                        lhsT=kt_j,
                                        rhs=qt_p[po:po + 64,
                                                 qoff + 128:qoff + QC],
                                        start=True, stop=True,
                                        skip_group_check=True)
                        PT = ptp.tile([128, 4 * QC], bf16, tag="PT",
                                      name=f"PT{c}_{e}_{js[0]}")
                        nc.scalar.activation(PT[:, :], SG[:, :], Exp,
                                             scale=0.125)
                        if js[-1] == 2 * c + 1:   # band group: mask on PT
                            for half in (0, 1):
                                off = 512 * half
                                nc.vector.tensor_tensor(
                                    PT[:, off:off + 128],
                                    PT[:, off:off + 128], tri_sb[:], mult)
                                nc.vector.tensor_tensor(
                                    PT[:, off + QC + 128:off + 2 * QC],
                                    PT[:, off + QC + 128:off + 2 * QC],
                                    tri_sb[:], mult)
                        flush_prev((c, e, js, PT))
                        for s in pts.get(idx, ()):
                            s()

                    # drain the lagged PV for this chunk's last group
                    flush_prev(None)
                    prev = None

                    # out-projection lagged two chunks behind its AG so
                    # the PE never waits on collective latency mid-stream
                    if c >= 2:
                        emit_outproj(c - 2)
                    # chunk complete on all 4 head pairs -> pairwise AG
                    nc.sync.dma_start(cc_in[c][:], xtiles[c][:])
                    nc.gpsimd.collective_compute(
                        "AllGather",
                        mybir.AluOpType.bypass,
                        replica_groups=[[0, 1], [2, 3], [4, 5], [6, 7]],
                        ins=[cc_in[c].opt()],
                        outs=[cc_out[c].opt()],
                    )
                emit_outproj(NCH - 2)
                emit_outproj(NCH - 1)
                if DEBUG_DUMP:
                    nc.sync.dma_start(dkt[:], kt_sb[:])
                    nc.sync.dma_start(dva[:], vaug_sb[:])

    nc.compile()
    return nc


def _program():
    global _PROG
    if _PROG is None:
        _PROG = _build_program()
    return _PROG


def _host_inputs(q, k, v, Wq, bq, Wk, bk, Wv, bv, Wo):
    qb = np.asarray(q, np.float32).astype(BF16)
    kb = np.asarray(k, np.float32).astype(BF16)
    vb = np.asarray(v, np.float32).astype(BF16)
    xqT = [np.ascontiguousarray(qb[b].T) for b in range(B)]
    xkT = [np.ascontiguousarray(kb[b].T) for b in range(B)]
    xvT = [np.ascontiguousarray(vb[b].T) for b in range(B)]

    def wslice(W, g):
        return np.ascontiguousarray(
            np.asarray(W, np.float32)[FG * g:FG * (g + 1), :].T).astype(BF16)

    wqg = [wslice(Wq, g) for g in range(2)]
    wkg = [wslice(Wk, g) for g in range(2)]
    wvg = [wslice(Wv, g) for g in range(2)]
    # AllGather output rows are rank-ordered (rank 0 = head-group 0 for
    # both cores of a pair), so woT rows stay in natural order.
    wog = [wslice(Wo, g) for g in range(2)]

    def bslice(bvec, g):
        return np.ascontiguousarray(
            np.asarray(bvec, np.float32)[FG * g:FG * (g + 1)]
            .reshape(4, 128).T)

    bqg = [bslice(bq, g) for g in range(2)]
    bkg = [bslice(bk, g) for g in range(2)]

    kk, qq = np.meshgrid(np.arange(128), np.arange(128), indexing="ij")
    tri = np.where(kk <= qq, 1.0, 0.0).astype(BF16)

    in_maps = []
    for core in range(N_CORES):
        b, g = core // 2, core % 2
        in_maps.append({
            "xqT": xqT[b], "xkT": xkT[b], "xvT": xvT[b],
            "wqT": wqg[g], "wkT": wkg[g], "wvT": wvg[g], "woT": wog[g],
            "bq2": bqg[g], "bk2": bkg[g], "tri01": tri,
        })
    return in_maps


def run_sharded(in_maps, trace=False, trace_kwargs=None):
    from concourse.bass_utils import run_bass_kernel_spmd
    nc = _program()
    return run_bass_kernel_spmd(nc, in_maps, core_ids=list(range(N_CORES)),
                                trace=trace, trace_kwargs=trace_kwargs or {})


def kernel(q, k, v, Wq, bq, Wk, bk, Wv, bv, Wo):
    in_maps = _host_inputs(q, k, v, Wq, bq, Wk, bk, Wv, bv, Wo)
    res = run_sharded(in_maps)
    out = np.empty((B, N, D), np.float32)
    for b in range(B):
        out[b, :, 0:FG] = np.asarray(res.results[2 * b]["y"], np.float32)
        out[b, :, FG:D] = np.asarray(res.results[2 * b + 1]["y"],
                                     np.float32)
    return out


# revision 39
# speedup vs baseline: 1.2212x; 1.0338x over previous
"""Trainium2 Bass kernel for nn_MultiHeadAttention_36051955483000.

Full-shape contract: kernel(**inputs) takes the complete fp32 tensors
(q,k,v: [4,2048,1024]; Wq/Wk/Wv/Wo: [1024,1024]; biases [1024]) and
returns the full [4,2048,1024] fp32 output.

Sharding (8 NeuronCores): core = 2*b + g for batch b in 0..3 and
head-group g in {0,1}. Each core computes 8 of the 16 heads for one
batch, AllGathers the attention output with its pair core, and runs
the output projection for its 512 output features.

Kernel structure (v4 — chunk-outer pipeline):
- The query-chunk loop (8 chunks of 256 queries) is OUTER; all 4 head
  pairs process a chunk before moving on. Q/K/V projections are
  demand-driven, emitted per chunk-PAIR at N=512 and interleaved into
  the previous chunks' attention stream: no serial projection phase.
- Scores are computed transposed (S^T, keys on partitions), head pairs
  ride disjoint PE row groups (contraction 64), exp(0.125 x) runs on
  the scalar engine, causal masking via tri-multiply on vector.
- PV matmuls use M=128: columns 0:64 of the stationary operand are the
  head's V, columns 64:128 are ones — output rows 64:128 come out as
  the softmax denominator broadcast across 64 partitions for free.
  The band block's dead front queries are skipped, not memset.
- Per chunk, once all 4 head pairs finish, the chunk's x^T is
  AllGathered pairwise; that chunk's out-projection runs two chunks
  later, fully overlapped. Only the last chunk's AG + out-proj trail.
- Queue discipline: collectives + their dependent cc_out loads live on
  the gpsimd queue (their waits block nothing else); DMA issues are
  few and large (dma_start costs ~0.7us of sequencer time each), with
  half-1 input loads deferred into chunks 0-2 so per-chunk AG copies
  are not stuck behind them in sync-queue FIFO order.
"""

import numpy as np
import ml_dtypes

B, N, D, H = 4, 2048, 1024, 16
DH = D // H            # 64
HG = H // 2            # 8 heads per core
FG = D // 2            # 512 features per head-group
N_CORES = 8
QC = 256               # query-chunk width
NCH = N // QC          # 8 chunks
NKB = N // 128         # 16 key blocks

BF16 = ml_dtypes.bfloat16

_PROG = None
DEBUG_DUMP = False


def _build_program():
    from concourse import bacc, tile, mybir

    f32 = mybir.dt.float32
    bf16 = mybir.dt.bfloat16

    nc = bacc.Bacc("TRN2", target_bir_lowering=False, debug=False,
                   num_devices=N_CORES)

    xqT = nc.dram_tensor("xqT", [D, N], bf16, kind="ExternalInput").ap()
    xkT = nc.dram_tensor("xkT", [D, N], bf16, kind="ExternalInput").ap()
    xvT = nc.dram_tensor("xvT", [D, N], bf16, kind="ExternalInput").ap()
    wqT = nc.dram_tensor("wqT", [D, FG], bf16, kind="ExternalInput").ap()
    wkT = nc.dram_tensor("wkT", [D, FG], bf16, kind="ExternalInput").ap()
    wvT = nc.dram_tensor("wvT", [D, FG], bf16, kind="ExternalInput").ap()
    woT = nc.dram_tensor("woT", [D, FG], bf16, kind="ExternalInput").ap()
    bq2 = nc.dram_tensor("bq2", [128, 4], f32, kind="ExternalInput").ap()
    bk2 = nc.dram_tensor("bk2", [128, 4], f32, kind="ExternalInput").ap()
    tri01 = nc.dram_tensor("tri01", [128, 128], bf16, kind="ExternalInput").ap()
    y = nc.dram_tensor("y", [N, FG], bf16, kind="ExternalOutput").ap()
    if DEBUG_DUMP:
        dkt = nc.dram_tensor("dkt", [128, 4 * N], bf16,
                             kind="ExternalOutput").ap()
        dva = nc.dram_tensor("dva", [128, NKB * HG * 128], bf16,
                             kind="ExternalOutput").ap()

    add = mybir.AluOpType.add
    mult = mybir.AluOpType.mult
    Exp = mybir.ActivationFunctionType.Exp

    with tile.TileContext(nc) as tc:
        with (
            tc.tile_pool(name="consts", bufs=1) as consts,
            tc.tile_pool(name="dram", bufs=1, space="DRAM") as dram,
            tc.tile_pool(name="xin", bufs=48) as xin,
        ):
            wq_sb = consts.tile([128, 8 * FG], bf16, tag="wq")
            wk_sb = consts.tile([128, 8 * FG], bf16, tag="wk")
            wv_sb = consts.tile([128, 8 * FG], bf16, tag="wv")
            wo_sb = consts.tile([128, 8 * FG], bf16, tag="wo")
            kt_sb = consts.tile([128, 4 * N], bf16, tag="kt")
            # per (kblock t, head h): 64 V columns then 64 ones columns;
            # PV with this 128-wide stationary operand emits the softmax
            # denominator broadcast over partitions 64:128 for free.
            vaug_sb = consts.tile([128, NKB * HG * 128], bf16, tag="vaug")
            bq_sb = consts.tile([128, 4], f32, tag="bq")
            bk_sb = consts.tile([128, 4], f32, tag="bk")
            tri_sb = consts.tile([128, 128], bf16, tag="tri")

            cc_in = [dram.tile([128, 4 * QC], bf16, name=f"cc_in{c}",
                               tag=f"cci{c}") for c in range(NCH)]
            cc_out = [dram.tile([256, 4 * QC], bf16, name=f"cc_out{c}",
                                tag=f"cco{c}") for c in range(NCH)]

            # Startup DMA split (dma_start costs ~0.7-1.5us of sequencer
            # issue time each): sync = consts + xq half 0; scalar = wq,
            # wk, xk half 0, wv, wo (in need order); gpsimd = xv half 0.
            # Half-1 input issues are deferred into chunks 0-2 so the
            # per-chunk AG copies aren't stuck behind them in FIFO.
            nc.sync.dma_start(bq_sb[:], bq2[:])
            nc.sync.dma_start(bk_sb[:], bk2[:])
            nc.sync.dma_start(tri_sb[:], tri01[:])

            xts = {}
            deferred_loads = {0: [], 1: [], 2: []}

            def make_xtile(nm, XT, db, half):
                t = xin.tile([128, N // 2], bf16, tag="xin",
                             name=f"x{nm}{db}_{half}")
                xts[(nm, db, half)] = t
                return t, XT[128 * db:128 * db + 128,
                             1024 * half:1024 * half + 1024]

            def load_w(W_sb, WT):
                for db in range(8):
                    nc.scalar.dma_start(W_sb[:, 512 * db:512 * db + 512],
                                        WT[128 * db:128 * db + 128, :])

            load_w(wq_sb, wqT)
            for db in range(8):
                t, src = make_xtile("q", xqT, db, 0)
                nc.sync.dma_start(t[:], src)
            load_w(wk_sb, wkT)
            for db in range(8):
                t, src = make_xtile("k", xkT, db, 0)
                nc.scalar.dma_start(t[:], src)
            for db in range(8):
                t, src = make_xtile("v", xvT, db, 0)
                nc.gpsimd.dma_start(t[:], src)
            load_w(wv_sb, wvT)
            load_w(wo_sb, woT)
            for nm, XT in (("q", xqT), ("k", xkT), ("v", xvT)):
                when = {"q": 0, "k": 1, "v": 2}[nm]
                for db in range(8):
                    deferred_loads[when].append(make_xtile(nm, XT, db, 1))

            vaug_v = vaug_sb[:, :].rearrange("p (t h c) -> p t h c",
                                             t=NKB, h=HG, c=128)
            nc.vector.memset(vaug_v[:, :, :, 64:128], 1.0)

            with (
                tc.tile_pool(name="mm", bufs=2, space="PSUM") as mmp,
                tc.tile_pool(name="sg", bufs=2, space="PSUM") as sgp,
                tc.tile_pool(name="otp", bufs=2, space="PSUM") as otp,
                tc.tile_pool(name="pt", bufs=2) as ptp,
                tc.tile_pool(name="ep", bufs=2) as ep,
                tc.tile_pool(name="xo", bufs=2) as xop,
                tc.tile_pool(name="yp", bufs=2) as yp,
                tc.tile_pool(name="qtp", bufs=2) as qtp,
                tc.tile_pool(name="xtp", bufs=2) as xtp,
            ):
                qtiles = {}      # chunk-pair -> [128, 4*512] Q^T tile
                xtiles = {}      # chunk -> [128, 4*QC] attention-out tile

                def x_slice(nm, db, lo, width):
                    # columns [lo, lo+width) of the full token axis
                    half = lo // 1024
                    assert (lo + width - 1) // 1024 == half
                    return xts[(nm, db, half)][:, lo - 1024 * half:
                                               lo - 1024 * half + width]

                def qk_set(nm, W_sb, bias, e, p):
                    # Q/K projection for head-pair e, chunk pair p
                    # (tokens [512p, 512p+512)), N=512
                    def emit():
                        if nm == "q" and p not in qtiles:
                            qtiles[p] = qtp.tile([128, 4 * 512], bf16,
                                                 tag="qt", name=f"qt{p}")
                        ps = mmp.tile([128, 512], f32, tag="mm",
                                      name=f"{nm}{e}_{p}")
                        for db in range(8):
                            nc.tensor.matmul(
                                ps[:],
                                lhsT=W_sb[:, 512 * db + 128 * e:
                                          512 * db + 128 * e + 128],
                                rhs=x_slice(nm, db, 512 * p, 512),
                                start=(db == 0), stop=(db == 7))
                        if nm == "q":
                            out_ap = qtiles[p][:, 512 * e:512 * e + 512]
                        else:
                            out_ap = kt_sb[:, 2048 * e + 512 * p:
                                           2048 * e + 512 * p + 512]
                        nc.vector.tensor_scalar(
                            out_ap, ps[:], bias[:, e:e + 1], None, add)
                    return emit

                def v_set(tb):
                    # V projection for key block tb (128 tokens), N=512
                    def emit():
                        ps = mmp.tile([128, 512], f32, tag="mm",
                                      name=f"v{tb}")
                        for db in range(8):
                            nc.tensor.matmul(
                                ps[:],
                                lhsT=x_slice("v", db, 128 * tb, 128),
                                rhs=wv_sb[:, 512 * db:512 * db + 512],
                                start=(db == 0), stop=(db == 7))
                        nc.vector.tensor_copy(
                            vaug_v[:, tb, :, 0:64],
                            ps[:, :].rearrange("p (h c) -> p h c",
                                               h=HG, c=64))
                    return emit

                def proj_sets_for_pair(p):
                    sets = []
                    for e in range(4):
                        sets.append(qk_set("q", wq_sb, bq_sb, e, p))
                        sets.append(qk_set("k", wk_sb, bk_sb, e, p))
                    for tb in range(4 * p, 4 * p + 4):
                        sets.append(v_set(tb))
                    return sets

                # prologue: just enough of pair 0 for wave A of chunk 0;
                # the rest is interleaved into chunk 0's stream.
                pair0 = proj_sets_for_pair(0)
                # order: q(e0) k(e0) q(e1) k(e1) v(kb0) v(kb1)
                for s in (pair0[0], pair0[1], pair0[2], pair0[3],
                          pair0[8], pair0[9]):
                    s()
                pair0_rest = [pair0[4], pair0[5], pair0[6], pair0[7],
                              pair0[10], pair0[11]]

                def emit_epilogue(e, c, OT2):
                    # OT2 [128, 2*QC]: rows 0:64 = O^T (head 2e cols 0:QC,
                    # head 2e+1 cols QC:2QC); rows 64:128 = denominators
                    # broadcast. Normalize and write x^T.
                    # reciprocal_approx_fast is a custom DVE op; feed it a
                    # partition-0-based tile (copy handles the offset).
                    dsb = ep.tile([64, 2 * QC], f32, tag="dsb",
                                  name=f"dsb{e}_{c}")
                    nc.vector.tensor_copy(dsb[:, :], OT2[64:128, :])
                    bcs = ep.tile([64, 2 * QC], f32, tag="bcs",
                                  name=f"bcs{e}_{c}")
                    nc.vector.reciprocal_approx_fast(bcs[:, :], dsb[:, :])
                    for half in (0, 1):
                        nc.vector.tensor_tensor(
                            xtiles[c][64 * half:64 * half + 64,
                                      QC * e:QC * e + QC],
                            OT2[0:64, QC * half:QC * half + QC],
                            bcs[:, QC * half:QC * half + QC], mult)

                def emit_outproj(c, eng=None):
                    # chunk c tokens: load gathered x blocks (gpsimd queue,
                    # right after that AG's wait -> never blocks compute;
                    # the tail outprojes load via sync so they don't sit
                    # behind the final AG's blocking wait), contract all
                    # 1024 features into 512 output columns.
                    eng = eng or nc.gpsimd
                    xta = xop.tile([128, 4 * QC], bf16, tag="xo",
                                   name=f"xta{c}")
                    xtb = xop.tile([128, 4 * QC], bf16, tag="xo",
                                   name=f"xtb{c}")
                    eng.dma_start(xta[:], cc_out[c][0:128, :])
                    eng.dma_start(xtb[:], cc_out[c][128:256, :])
                    for tb in range(2):
                        ps = mmp.tile([128, 512], f32, tag="mm",
                                      name=f"op{c}_{tb}")
                        i = 0
                        for xt in (xta, xtb):
                            for e in range(4):
                                nc.tensor.matmul(
                                    ps[:],
                                    lhsT=xt[:, QC * e + 128 * tb:
                                            QC * e + 128 * tb + 128],
                                    rhs=wo_sb[:, 512 * i:512 * i + 512],
                                    start=(i == 0), stop=(i == 7))
                                i += 1
                        ysb = yp.tile([128, 512], bf16, tag="ysb",
                                      name=f"y{c}_{tb}")
                        nc.vector.tensor_copy(ysb[:], ps[:])
                        nc.sync.dma_start(
                            y[QC * c + 128 * tb:QC * c + 128 * tb + 128, :],
                            ysb[:])

                # ---- main chunk-outer attention stream ----
                prev = None          # (c, e, js, PT) lagged one group
                ots = {}             # (c, e) -> OT2 psum tile

                def flush_prev(nxt):
                    nonlocal prev
                    if prev is None:
                        prev = nxt
                        return
                    pc, pe, pjs, pPT = prev
                    pOT2 = ots[(pc, pe)]
                    for m, j in enumerate(pjs):
                        for half in (0, 1):
                            if j == 2 * pc + 1:
                                # band block: front 128 queries are dead
                                # (never written) — stream only the back
                                rhs = pPT[:, 512 * half + QC * m + 128:
                                          512 * half + QC * m + QC]
                                out = pOT2[:, QC * half + 128:
                                           QC * half + QC]
                            else:
                                rhs = pPT[:, 512 * half + QC * m:
                                          512 * half + QC * m + QC]
                                out = pOT2[:, QC * half:QC * half + QC]
                            nc.tensor.matmul(
                                out, lhsT=vaug_v[:, j, 2 * pe + half, :],
                                rhs=rhs,
                                # one start per PSUM bank: start clears
                                # has_written bank-wide
                                start=(j == 0 and half == 0),
                                stop=(j == 2 * pc + 1),
                                skip_group_check=True)
                    if pjs[-1] == 2 * pc + 1:     # (pc, pe) complete
                        emit_epilogue(pe, pc, pOT2)
                        del ots[(pc, pe)]
                    prev = nxt

                for c in range(NCH):
                    xtiles[c] = xtp.tile([128, 4 * QC], bf16, tag="xt",
                                         name=f"xt{c}")
                    # deferred half-1 input loads (sync queue)
                    for t, src in deferred_loads.pop(c, ()):
                        nc.sync.dma_start(t[:], src)
                    # projection sets to interleave into this chunk: pair
                    # p = chunks (2p, 2p+1), emitted over chunks 2p-2/2p-1
                    pn = []
                    if c == 0:
                        pn = proj_sets_for_pair(1)[0:6]
                    elif c <= 5:
                        p = c // 2 + 1
                        pn = (proj_sets_for_pair(p)[0:6] if c % 2 == 0
                              else proj_sets_for_pair(p)[6:12])
                    groups = []
                    for wave in ((0, 1), (2, 3)):
                        for gi in range(c + 1):
                            for e in wave:
                                groups.append((e, gi))
                    pts = {}
                    if c == 0:
                        # deadline-aware: group (e,0) consumes q/k(e) —
                        # those sets must precede it in PE queue order
                        pts = {0: pair0_rest[0:2],    # q/k(e2) before grp 2
                               1: pair0_rest[2:4],    # q/k(e3) before grp 3
                               3: pair0_rest[4:6] + pn}
                    elif pn:
                        step = max(1, len(groups) // len(pn))
                        for i in range(len(pn)):
                            pts.setdefault(min((i + 1) * step,
                                               len(groups) - 1), []).append(
                                pn[i])

                    for idx, (e, gi) in enumerate(groups):
                        js = [2 * gi, 2 * gi + 1]
                        if gi == 0 and (c, e) not in ots:
                            ots[(c, e)] = otp.tile(
                                [128, 2 * QC], f32, tag="OT2",
                                name=f"OT2_{c}_{e}")
                        SG = sgp.tile([128, 4 * QC], f32, tag="SG",
                                      name=f"SG{c}_{e}_{js[0]}")
                        qt_p = qtiles[c // 2]
                        qoff = 512 * e + QC * (c % 2)
                        for m, j in enumerate(js):
                            for half in (0, 1):
                                po = 64 * half
                                off = 512 * half + QC * m
                                kt_j = kt_sb[po:po + 64,
                                             2048 * e + 128 * j:
                                             2048 * e + 128 * j + 128]
                                if j <= 2 * c:
                                    nc.tensor.matmul(
                                        SG[:, off:off + QC], lhsT=kt_j,
                                        rhs=qt_p[po:po + 64,
                                                 qoff:qoff + QC],
                                        start=True, stop=True,
                                        skip_group_check=True)
                                else:   # j == 2c+1: front half is dead
                                    nc.tensor.matmul(
                                        SG[:, off + 128:off + QC],
                                        lhsT=kt_j,
                                        rhs=qt_p[po:po + 64,
                                                 qoff + 128:qoff + QC],
                                        start=True, stop=True,
                                        skip_group_check=True)
                        PT = ptp.tile([128, 4 * QC], bf16, tag="PT",
                                      name=f"PT{c}_{e}_{js[0]}")
                        nc.scalar.activation(PT[:, :], SG[:, :], Exp,
                                             scale=0.125)
                        if js[-1] == 2 * c + 1:   # band group: mask on PT
                            for half in (0, 1):
                                off = 512 * half
                                nc.vector.tensor_tensor(
                                    PT[:, off:off + 128],
                                    PT[:, off:off + 128], tri_sb[:], mult)
                                nc.vector.tensor_tensor(
                                    PT[:, off + QC + 128:off + 2 * QC],
                                    PT[:, off + QC + 128:off + 2 * QC],
                                    tri_sb[:], mult)
                        flush_prev((c, e, js, PT))
                        for s in pts.get(idx, ()):
                            s()

                    # drain the lagged PV for this chunk's last group
                    flush_prev(None)
                    prev = None

                    # out-projection lagged two chunks behind its AG so
                    # the PE never waits on collective latency mid-stream
                    if c >= 2:
                        emit_outproj(c - 2)
                    # chunk complete on all 4 head pairs -> pairwise AG
                    nc.sync.dma_start(cc_in[c][:], xtiles[c][:])
                    nc.gpsimd.collective_compute(
                        "AllGather",
                        mybir.AluOpType.bypass,
                        replica_groups=[[0, 1], [2, 3], [4, 5], [6, 7]],
                        ins=[cc_in[c].opt()],
                        outs=[cc_out[c].opt()],
                    )
                emit_outproj(NCH - 2, eng=nc.sync)
                emit_outproj(NCH - 1, eng=nc.sync)
                if DEBUG_DUMP:
                    nc.sync.dma_start(dkt[:], kt_sb[:])
                    nc.sync.dma_start(dva[:], vaug_sb[:])

    nc.compile()
    return nc


def _program():
    global _PROG
    if _PROG is None:
        _PROG = _build_program()
    return _PROG


def _host_inputs(q, k, v, Wq, bq, Wk, bk, Wv, bv, Wo):
    qb = np.asarray(q, np.float32).astype(BF16)
    kb = np.asarray(k, np.float32).astype(BF16)
    vb = np.asarray(v, np.float32).astype(BF16)
    xqT = [np.ascontiguousarray(qb[b].T) for b in range(B)]
    xkT = [np.ascontiguousarray(kb[b].T) for b in range(B)]
    xvT = [np.ascontiguousarray(vb[b].T) for b in range(B)]

    def wslice(W, g):
        return np.ascontiguousarray(
            np.asarray(W, np.float32)[FG * g:FG * (g + 1), :].T).astype(BF16)

    wqg = [wslice(Wq, g) for g in range(2)]
    wkg = [wslice(Wk, g) for g in range(2)]
    wvg = [wslice(Wv, g) for g in range(2)]
    # AllGather output rows are rank-ordered (rank 0 = head-group 0 for
    # both cores of a pair), so woT rows stay in natural order.
    wog = [wslice(Wo, g) for g in range(2)]

    def bslice(bvec, g):
        return np.ascontiguousarray(
            np.asarray(bvec, np.float32)[FG * g:FG * (g + 1)]
            .reshape(4, 128).T)

    bqg = [bslice(bq, g) for g in range(2)]
    bkg = [bslice(bk, g) for g in range(2)]

    kk, qq = np.meshgrid(np.arange(128), np.arange(128), indexing="ij")
    tri = np.where(kk <= qq, 1.0, 0.0).astype(BF16)

    in_maps = []
    for core in range(N_CORES):
        b, g = core // 2, core % 2
        in_maps.append({
            "xqT": xqT[b], "xkT": xkT[b], "xvT": xvT[b],
            "wqT": wqg[g], "wkT": wkg[g], "wvT": wvg[g], "woT": wog[g],
            "bq2": bqg[g], "bk2": bkg[g], "tri01": tri,
        })
    return in_maps


def run_sharded(in_maps, trace=False, trace_kwargs=None):
    from concourse.bass_utils import run_bass_kernel_spmd
    nc = _program()
    return run_bass_kernel_spmd(nc, in_maps, core_ids=list(range(N_CORES)),
                                trace=trace, trace_kwargs=trace_kwargs or {})


def kernel(q, k, v, Wq, bq, Wk, bk, Wv, bv, Wo):
    in_maps = _host_inputs(q, k, v, Wq, bq, Wk, bk, Wv, bv, Wo)
    res = run_sharded(in_maps)
    out = np.empty((B, N, D), np.float32)
    for b in range(B):
        out[b, :, 0:FG] = np.asarray(res.results[2 * b]["y"], np.float32)
        out[b, :, FG:D] = np.asarray(res.results[2 * b + 1]["y"],
                                     np.float32)
    return out
